# revision 7
# baseline (speedup 1.0000x reference)
"""Trainium2 Bass kernel for nn_AttentionModel_63737314672806.

Sharding: data-parallel over batch (B=128) across 8 NeuronCores; each core
processes 16 batch elements (2048 tokens) through the full model. Weights are
replicated (broadcast) to every core. No collectives.

Device layout: activations are kept feature-major ("transposed"):
  xT[p, c, t] = x[token t, feature c*128+p]   (SBUF tile [128, 8, 2048])
so every dense layer is psum[dout, tok] = sum_kc matmul(lhsT=W[kc, dout_chunk],
rhs=xT[kc, tok_tile]) and the output is feature-major again (no transposes).
LayerNorm reductions over features (partition dim) use ones-vector matmuls;
per-token scalars are broadcast along partitions with rank-1 matmuls.
Softmax is computed in the transposed attention layout awT[k, q] (which falls
out of matmul(lhsT=kT, rhs=qT)) so no transposes are needed in attention
either; the relative-position bias is applied as exp(logit)*exp(bias) with a
host-precomputed exp-table.

Precision: the large projections/FFN matmuls run in fp8 e4m3 with
perf_mode=DoubleRow (2 fp8 weights per PE cell -> 2x throughput). Weights are
host-quantized with power-of-2 per-matrix scales (SW=1024, SW1=32 for Wf1);
descales fold into existing epilogue constants. Residual, LayerNorm and
pooling paths stay bf16/fp32: fp8 copies of x and h are produced on the
(otherwise idle) gpsimd engine purely as matmul inputs. Attention q/k/v/aw
are fp8 (q pre-scaled by 32 so layer-0 values clear the fp8 subnormal range;
the 1/32 folds into the exp's scale operand). Softmax/LN statistics are fp32.

Measured (numpy emulation of this exact quantization): rel-l2 4.2e-3 vs the
fp32 reference.
"""

import math

import numpy as np
import ml_dtypes

import concourse.bass as bass
import concourse.bacc as bacc
import concourse.mybir as mybir
import concourse.tile as tile
from concourse.bass_utils import run_bass_kernel_spmd

BF16 = mybir.dt.bfloat16
F8 = mybir.dt.float8e4
F32 = mybir.dt.float32
F32R = mybir.dt.float32r
AF = mybir.ActivationFunctionType
OP = mybir.AluOpType
DR = mybir.MatmulPerfMode.DoubleRow

NCORES = 8
B = 128
L = 128
DFEAT = 32
H = 8
DK = 128
D = 1024  # = H * DK
FF = 4096
NL = 2
MAXPOS = 128
OTHER = 64
EPS = 1e-6

BPC = B // NCORES       # 16 batches per core
NTOK = BPC * L          # 2048 tokens per core
NG = 4                  # batch groups per core (4 batches / 512 tokens each)
GB = BPC // NG          # batches per group = 4
GT = GB * L             # tokens per group = 512
DC = D // 128           # 8 feature chunks
FC = FF // 128          # 32 ff chunks
QSCALE = 1.0 / math.sqrt(float(DK))
SURVIVE = [1.0, 0.5]    # jnp.linspace(1.0, 0.5, 2)

# fp8 scale plan (all powers of 2; folded into epilogue constants)
SW = 1024.0             # weight scale: wq wk wv wg wo wfg wf2
SW1 = 32.0              # weight scale for wf1 (its descale rides on f8)
SQ = 32.0               # q stored as 32*q
SV = 32.0               # v stored as 32*v
SAO = 32.0              # ao stored as 32*ao  (SV==SAO makes rb == 1/sm)
SF = SW1                # f stored as 32*f

CQ = SQ * QSCALE / SW
CK = 1.0 / SW
CV = SV / SW
CO = 1.0 / (SW * SAO)
CG = 1.0 / SW
CFG = 1.0 / SW
CF2 = 1.0 / (SW * SF)
EXPS = 1.0 / SQ

_cached = {}


def _build_nc():
    nc = bacc.Bacc("TRN2", target_bir_lowering=False, debug=False,
                   num_devices=NCORES)

    def din(name, shape, dtype):
        return nc.dram_tensor(name, list(shape), dtype, kind="ExternalInput")

    t = {}
    t["cgmT"] = din("cgmT", [DFEAT, NTOK], BF16)
    t["w_in"] = din("w_in", [DFEAT, D], BF16)
    t["b_in_c"] = din("b_in_c", [128, DC], F32)
    for w in ("wq", "wk", "wv", "wg", "wo"):
        t[w] = din(w, [NL, 128, DC, D], F8)
    t["wf1"] = din("wf1", [NL, 128, DC, FF], F8)
    t["wfg"] = din("wfg", [NL, 128, DC, FF], F8)
    t["wf2"] = din("wf2", [NL, 128, FC, D], F8)
    for bn in ("bq_c", "bk_c", "bg_c", "bo_c", "bf2_c",
               "ln1_s_c", "ln1_b_c", "ln2_s_c", "ln2_b_c"):
        t[bn] = din(bn, [128, NL, DC], F32)
    t["bf1_c"] = din("bf1_c", [128, NL, FC], F32)
    t["bfg_c"] = din("bfg_c", [128, NL, FC], F32)
    t["ln1_s_row"] = din("ln1_s_row", [1, NL, D], F32R)
    t["ln2_s_row"] = din("ln2_s_row", [1, NL, D], F32R)
    t["bv_row"] = din("bv_row", [1, NL, D], BF16)
    t["exptab"] = din("exptab", [128, NL, 143], BF16)
    t["wd1"] = din("wd1", [128, 17, 128], BF16)
    t["bd1_c"] = din("bd1_c", [128, 1], F32)
    t["ln3_s_c"] = din("ln3_s_c", [128, 1], F32)
    t["ln3_b_c"] = din("ln3_b_c", [128, 1], F32)
    t["ln3_s_row"] = din("ln3_s_row", [1, 128], F32R)
    t["wd2"] = din("wd2", [128, 128], BF16)
    t["bd2_c"] = din("bd2_c", [128, 1], F32)
    t["wout"] = din("wout", [128, 1], BF16)
    t["bout_t"] = din("bout_t", [1, 1], F32)
    t["otherT"] = din("otherT", [128, BPC], BF16)
    t["onesc"] = din("onesc", [128, 1], F32R)
    t["onesr"] = din("onesr", [1, 128], F32R)
    y_out = nc.dram_tensor("y", [1, BPC], F32, kind="ExternalOutput")

    with tile.TileContext(nc, pool_alloc_mode="queue") as tc:
        _emit(nc, tc, t, y_out)
    nc.compile()
    return nc


def _emit(nc, tc, t, y_out):
    with (
        tc.tile_pool(name="persist", bufs=1) as pp,
        tc.tile_pool(name="dramp", bufs=1, space="DRAM") as dp,
        tc.tile_pool(name="mm_psum", bufs=6, space="PSUM") as mmp,
        tc.tile_pool(name="stat_psum", bufs=1, space="PSUM") as stp,
    ):
        # ---- persistent SBUF state ----
        xT = pp.tile([128, DC, NTOK], BF16)
        xT8 = pp.tile([128, DC, NTOK], F8)
        h8 = pp.tile([128, DC, NTOK], F8)
        ones_col_bf = pp.tile([128, 1], BF16)
        nc.vector.memset(ones_col_bf, 1.0)
        ones_col_8 = pp.tile([128, 1], F8)
        nc.vector.memset(ones_col_8, 1.0)
        ones_row_bf = pp.tile([1, 128], BF16)
        nc.vector.memset(ones_row_bf, 1.0)
        ones_col_f = pp.tile([128, 1], F32R)
        nc.sync.dma_start(out=ones_col_f[:], in_=t["onesc"][:])
        ones_row_f = pp.tile([1, 128], F32R)
        nc.sync.dma_start(out=ones_row_f[:], in_=t["onesr"][:])
        eps1 = pp.tile([1, 1], F32)
        nc.vector.memset(eps1, EPS)

        # input projection operands first: the first matmuls need them
        early = {}
        for name in ("cgmT", "w_in"):
            ap = t[name]
            tl = pp.tile(list(ap.shape), ap.dtype, name=f"e_{name}")
            nc.sync.dma_start(out=tl[:], in_=ap[:])
            early[name] = tl

        # small constants from DRAM
        consts = {}
        for name in ("b_in_c", "bq_c", "bk_c", "bg_c", "bo_c", "bf2_c",
                     "ln1_s_c", "ln1_b_c", "ln2_s_c", "ln2_b_c",
                     "bf1_c", "bfg_c", "ln1_s_row", "ln2_s_row", "bv_row",
                     "exptab", "bd1_c", "ln3_s_c", "ln3_b_c",
                     "ln3_s_row", "wd2", "bd2_c", "wout", "bout_t", "otherT"):
            ap = t[name]
            tl = pp.tile(list(ap.shape), ap.dtype, name=f"c_{name}")
            nc.sync.dma_start(out=tl[:], in_=ap[:])
            consts[name] = tl

        # ---- input projection: xT = cgm @ W_in + b_in ----
        cgmT_s = early["cgmT"]
        w_in_s = early["w_in"]
        for dd in range(DC):
            for g in range(NG):
                ps = mmp.tile([128, GT], F32, tag="mm")
                nc.tensor.matmul(ps[:], w_in_s[:, dd * 128:(dd + 1) * 128],
                                 cgmT_s[:, g * GT:(g + 1) * GT],
                                 start=True, stop=True)
                nc.vector.tensor_scalar_add(
                    out=xT[:, dd, g * GT:(g + 1) * GT], in0=ps[:],
                    scalar1=consts["b_in_c"][:, dd:dd + 1])
                nc.gpsimd.tensor_copy(
                    out=xT8[:, dd, g * GT:(g + 1) * GT],
                    in_=xT[:, dd, g * GT:(g + 1) * GT])

        h_dram = dp.tile([128, DC, NTOK], BF16)

        # ---- transformer layers ----
        for i in range(NL):
            _layer_attn(nc, tc, t, consts, i, xT, xT8, h8, h_dram,
                        ones_col_bf, ones_col_8, ones_row_bf, ones_col_f,
                        ones_row_f, eps1, mmp, stp)
            _layer_ffn(nc, tc, t, consts, i, xT, xT8, h8, h_dram,
                       ones_col_f, ones_row_f, eps1, mmp, stp)

        # ---- head ----
        _head(nc, tc, t, consts, xT, ones_col_f, ones_row_f, eps1, mmp, stp,
              y_out)


def _ln_rows(nc, lp, psum_s, psum_q, n, inv_d, eps1):
    """From psum_s=sum(x) and psum_q=sum(x^2) over features ([1, n] each),
    compute rs_row = 1/sqrt(var+eps) and mrs_row = mean*rs ([1, n] f32)."""
    m_row = lp.tile([1, 512], F32R, tag="m", name="m_row")[:, :n]
    with nc.allow_low_precision(reason="fp32r LN stats within tolerance"):
        nc.vector.tensor_scalar_mul(out=m_row, in0=psum_s[:], scalar1=inv_d)
    m_f = m_row.bitcast(F32)
    ex2 = lp.tile([1, 512], F32, tag="e", name="ex2")[:, :n]
    nc.vector.tensor_scalar_mul(out=ex2, in0=psum_q[:], scalar1=inv_d)
    var = lp.tile([1, 512], F32, tag="v", name="var")[:, :n]
    # var = ex2 - m*m
    nc.vector.scalar_tensor_tensor(out=var, in0=m_f, scalar=-1.0, in1=m_f,
                                   op0=OP.mult, op1=OP.mult)
    nc.vector.tensor_add(out=var, in0=var, in1=ex2)
    nc.scalar.activation(out=var, in_=var, func=AF.Sqrt, bias=eps1[:],
                         scale=1.0)
    rs_row = lp.tile([1, 512], F32R, tag="r", name="rs_row")[:, :n]
    with nc.allow_low_precision(reason="fp32r LN stats within tolerance"):
        nc.vector.reciprocal(out=rs_row, in_=var)
        nc.vector.tensor_mul(out=m_row, in0=m_f, in1=rs_row.bitcast(F32))
    return rs_row, m_row


def _layer_attn(nc, tc, t, consts, i, xT, xT8, h8, h_dram, ones_col_bf,
                ones_col_8, ones_row_bf, ones_col_f, ones_row_f, eps1, mmp,
                stp):
    with (
        tc.tile_pool(name="wqk", bufs=1) as wp,
        tc.tile_pool(name="grp", bufs=2) as gp,
        tc.tile_pool(name="wch", bufs=2) as wc,
        tc.tile_pool(name="wvch", bufs=2) as wvc,
        tc.tile_pool(name="aog", bufs=1) as aop,
        tc.tile_pool(name="att", bufs=2) as at,
        tc.tile_pool(name="res", bufs=1) as rp,
        tc.tile_pool(name="sq", bufs=2) as sqp,
        tc.tile_pool(name="upool", bufs=1) as up,
        tc.tile_pool(name="lnsm", bufs=1) as lp,
        tc.tile_pool(name="hst", bufs=1) as hstp,
    ):
        wq_s = wp.tile([128, DC, D], F8)
        nc.sync.dma_start(out=wq_s[:], in_=t["wq"][i])
        wk_s = wp.tile([128, DC, D], F8)
        nc.sync.dma_start(out=wk_s[:], in_=t["wk"][i])

        for g in range(NG):
            tok = slice(g * GT, (g + 1) * GT)
            # --- Q/K projections (feature-major; q carries SQ*QSCALE) ---
            qT_g = gp.tile([128, DC, GT], F8, tag="q")
            kT_g = gp.tile([128, DC, GT], F8, tag="k")
            for dd in range(DC):
                psq = mmp.tile([128, GT], F32, tag="mm")
                psk = mmp.tile([128, GT], F32, tag="mm")
                for j in range(DC // 2):
                    nc.tensor.matmul(
                        psq[:], wq_s[:, 2 * j:2 * j + 2,
                                     dd * 128:(dd + 1) * 128],
                        xT8[:, 2 * j:2 * j + 2, tok],
                        start=(j == 0), stop=(j == DC // 2 - 1), perf_mode=DR)
                for j in range(DC // 2):
                    nc.tensor.matmul(
                        psk[:], wk_s[:, 2 * j:2 * j + 2,
                                     dd * 128:(dd + 1) * 128],
                        xT8[:, 2 * j:2 * j + 2, tok],
                        start=(j == 0), stop=(j == DC // 2 - 1), perf_mode=DR)
                nc.vector.tensor_scalar(
                    out=qT_g[:, dd, :], in0=psq[:], scalar1=CQ,
                    scalar2=consts["bq_c"][:, i, dd:dd + 1],
                    op0=OP.mult, op1=OP.add)
                nc.vector.tensor_scalar(
                    out=kT_g[:, dd, :], in0=psk[:], scalar1=CK,
                    scalar2=consts["bk_c"][:, i, dd:dd + 1],
                    op0=OP.mult, op1=OP.add)

            # --- V projection (token-major, stored as SV*v) ---
            v_g = gp.tile([128, GB, D], F8, tag="v")
            for cc in range(2):
                wv_ch = wvc.tile([128, DC, 512], F8, tag="wv")
                nc.gpsimd.dma_start(out=wv_ch[:],
                                  in_=t["wv"][i, :, :, cc * 512:(cc + 1) * 512])
                for jj in range(GB):
                    btok = slice((g * GB + jj) * L, (g * GB + jj + 1) * L)
                    psv = mmp.tile([128, 512], F32, tag="mm")
                    for j in range(DC // 2):
                        nc.tensor.matmul(psv[:],
                                         xT8[:, 2 * j:2 * j + 2, btok],
                                         wv_ch[:, 2 * j:2 * j + 2, :],
                                         start=(j == 0), stop=False,
                                         perf_mode=DR)
                    nc.tensor.matmul(psv[:], ones_row_bf[:],
                                     consts["bv_row"][:, i,
                                                      cc * 512:(cc + 1) * 512],
                                     start=False, stop=True)
                    nc.scalar.activation(
                        out=v_g[:, jj, cc * 512:(cc + 1) * 512], in_=psv[:],
                        func=AF.Copy, scale=CV)

            # --- attention per (batch, head) ---
            ao_g = aop.tile([128, DC, GT], F8, tag="ao")
            for jj in range(GB):
                b_local = g * GB + jj
                jtok = slice(jj * L, (jj + 1) * L)
                etab = consts["exptab"][:, i, 15 - b_local:143 - b_local]
                for hh in range(H):
                    pa = mmp.tile([128, 512], F32, tag="mm", name="pa")
                    aw = pa[:, 0:128]
                    ao = pa[:, 256:384]
                    sm = pa[0:1, 384:512]
                    nc.tensor.matmul(aw, kT_g[:, hh, jtok], qT_g[:, hh, jtok],
                                     start=True, stop=True)
                    awe = at.tile([128, 128], BF16, tag="awe")
                    nc.scalar.activation(out=awe[:], in_=aw, func=AF.Exp,
                                         scale=EXPS)
                    awe2 = at.tile([128, 128], F8, tag="awe2")
                    nc.vector.tensor_mul(out=awe2[:], in0=awe[:], in1=etab)
                    nc.tensor.matmul(sm, ones_col_8[:, 0:1], awe2[:, 0:128],
                                     start=True, stop=True)
                    rc = at.tile([1, 128], BF16, tag="rc")
                    with nc.allow_low_precision(
                            reason="bf16 softmax normalizer is within tolerance"):
                        nc.vector.reciprocal(out=rc[:], in_=sm)
                    rb = at.tile([128, 128], BF16, tag="rb")
                    nc.gpsimd.partition_broadcast(rb[:], rc[:])
                    nc.tensor.matmul(ao, v_g[:, jj, hh * 128:(hh + 1) * 128],
                                     awe2[:], start=True, stop=True)
                    nc.vector.tensor_mul(out=ao_g[:, hh, jtok], in0=ao,
                                         in1=rb[:])

            # --- o-proj + gate + residual + LN1 stats ---
            res_t = rp.tile([128, DC, GT], BF16, tag="res")
            ps_s = stp.tile([1, GT], F32, tag="s")
            ps_q = stp.tile([1, GT], F32, tag="q")
            for dd in range(DC):
                wo_ch = wc.tile([128, DC, 128], F8, tag="wo")
                nc.sync.dma_start(out=wo_ch[:],
                                  in_=t["wo"][i, :, :, dd * 128:(dd + 1) * 128])
                wg_ch = wc.tile([128, DC, 128], F8, tag="wg")
                nc.sync.dma_start(out=wg_ch[:],
                                  in_=t["wg"][i, :, :, dd * 128:(dd + 1) * 128])
                pso = mmp.tile([128, GT], F32, tag="mm")
                psg = mmp.tile([128, GT], F32, tag="mm")
                for j in range(DC // 2):
                    nc.tensor.matmul(pso[:], wo_ch[:, 2 * j:2 * j + 2, :],
                                     ao_g[:, 2 * j:2 * j + 2, :],
                                     start=(j == 0), stop=(j == DC // 2 - 1),
                                     perf_mode=DR)
                for j in range(DC // 2):
                    nc.tensor.matmul(psg[:], wg_ch[:, 2 * j:2 * j + 2, :],
                                     xT8[:, 2 * j:2 * j + 2, tok],
                                     start=(j == 0), stop=(j == DC // 2 - 1),
                                     perf_mode=DR)
                sig = sqp.tile([128, GT], BF16, tag="sig")
                nc.scalar.activation(out=sig[:], in_=psg[:], func=AF.Sigmoid,
                                     bias=consts["bg_c"][:, i, dd:dd + 1],
                                     scale=CG)
                ot = sqp.tile([128, GT], BF16, tag="ot")
                nc.scalar.activation(out=ot[:], in_=pso[:],
                                     func=AF.Identity,
                                     bias=consts["bo_c"][:, i, dd:dd + 1],
                                     scale=CO)
                # res = x + sig * (o + bo)
                nc.vector.tensor_mul(out=res_t[:, dd, :], in0=ot[:],
                                     in1=sig[:])
                nc.vector.tensor_add(out=res_t[:, dd, :],
                                     in0=res_t[:, dd, :], in1=xT[:, dd, tok])
                sq = sqp.tile([128, GT], BF16, tag="sq")
                nc.scalar.activation(out=sq[:], in_=res_t[:, dd, :],
                                     func=AF.Square)
                nc.tensor.matmul(ps_s[:], ones_col_bf[:, 0:1],
                                 res_t[:, dd, :],
                                 start=(dd == 0), stop=(dd == DC - 1))
                nc.tensor.matmul(ps_q[:], ones_col_bf[:, 0:1],
                                 sq[:],
                                 start=(dd == 0), stop=(dd == DC - 1))

            # --- LN1 apply -> h (bf16 to DRAM for the residual; fp8 copy
            # in SBUF for the FFN matmuls) ---
            rs_row, mrs_row = _ln_rows(nc, lp, ps_s, ps_q, GT, 1.0 / D, eps1)
            brs = mmp.tile([128, GT], F32, tag="mm")
            nc.tensor.matmul(brs[:], ones_row_f[:],
                             rs_row, start=True, stop=True)
            h_g = hstp.tile([128, DC, GT], BF16, tag="hg")
            for dd in range(DC):
                bms = mmp.tile([128, GT], F32, tag="mm")
                nc.tensor.matmul(
                    bms[:],
                    consts["ln1_s_row"][:, i, dd * 128:(dd + 1) * 128],
                    mrs_row, start=True, stop=True)
                u = up.tile([128, GT], F32, tag="u")
                nc.vector.scalar_tensor_tensor(
                    out=u[:], in0=res_t[:, dd, :],
                    scalar=consts["ln1_s_c"][:, i, dd:dd + 1], in1=brs[:],
                    op0=OP.mult, op1=OP.mult)
                nc.vector.scalar_tensor_tensor(
                    out=h_g[:, dd, :], in0=u[:],
                    scalar=consts["ln1_b_c"][:, i, dd:dd + 1], in1=bms[:],
                    op0=OP.add, op1=OP.subtract)
            nc.gpsimd.tensor_copy(out=h8[:, :, tok], in_=h_g[:])
            nc.sync.dma_start(out=h_dram[:, :, tok], in_=h_g[:])


def _layer_ffn(nc, tc, t, consts, i, xT, xT8, h8, h_dram, ones_col_f,
               ones_row_f, eps1, mmp, stp):
    with (
        tc.tile_pool(name="fbuf", bufs=1) as fp,
        tc.tile_pool(name="fwch", bufs=2) as wc,
        tc.tile_pool(name="fres", bufs=1) as rp,
        tc.tile_pool(name="fsq", bufs=2) as sqp,
        tc.tile_pool(name="flnsm", bufs=1) as lp,
        tc.tile_pool(name="hsup", bufs=1) as hp2,
    ):
        for sup in range(2):
            f_s = fp.tile([128, FC, 1024], F8, tag="f")
            stok0 = sup * 1024
            h_s = hp2.tile([128, DC, 1024], BF16, tag="hs")
            nc.sync.dma_start(out=h_s[:],
                              in_=h_dram[:, :, stok0:stok0 + 1024])
            # --- f = SF * (h@Wf1 + bf1) * sigmoid(h@Wfg + bfg) ---
            for fc in range(FC):
                wf1_ch = wc.tile([128, DC, 128], F8, tag="wf1")
                nc.gpsimd.dma_start(out=wf1_ch[:],
                                  in_=t["wf1"][i, :, :, fc * 128:(fc + 1) * 128])
                wfg_ch = wc.tile([128, DC, 128], F8, tag="wfg")
                nc.gpsimd.dma_start(out=wfg_ch[:],
                                  in_=t["wfg"][i, :, :, fc * 128:(fc + 1) * 128])
                for sub in range(2):
                    tok8 = slice(stok0 + sub * 512, stok0 + (sub + 1) * 512)
                    ps1 = mmp.tile([128, 512], F32, tag="mm")
                    psg = mmp.tile([128, 512], F32, tag="mm")
                    for j in range(DC // 2):
                        nc.tensor.matmul(ps1[:], wf1_ch[:, 2 * j:2 * j + 2, :],
                                         h8[:, 2 * j:2 * j + 2, tok8],
                                         start=(j == 0),
                                         stop=(j == DC // 2 - 1), perf_mode=DR)
                    for j in range(DC // 2):
                        nc.tensor.matmul(psg[:], wfg_ch[:, 2 * j:2 * j + 2, :],
                                         h8[:, 2 * j:2 * j + 2, tok8],
                                         start=(j == 0),
                                         stop=(j == DC // 2 - 1), perf_mode=DR)
                    sig = sqp.tile([128, 512], BF16, tag="fsig")
                    nc.scalar.activation(out=sig[:], in_=psg[:],
                                         func=AF.Sigmoid,
                                         bias=consts["bfg_c"][:, i, fc:fc + 1],
                                         scale=CFG)
                    nc.vector.scalar_tensor_tensor(
                        out=f_s[:, fc, sub * 512:(sub + 1) * 512], in0=ps1[:],
                        scalar=consts["bf1_c"][:, i, fc:fc + 1], in1=sig[:],
                        op0=OP.add, op1=OP.mult)

            # --- f @ Wf2 + bf2 + h, then LN2 + stochastic-depth residual ---
            for sub in range(2):
                tok = slice(stok0 + sub * 512, stok0 + (sub + 1) * 512)
                res_t = rp.tile([128, DC, 512], F32R, tag="res2")
                ps_s = stp.tile([1, 512], F32, tag="s")
                ps_q = stp.tile([1, 512], F32, tag="q")
                for dd in range(DC):
                    wf2_ch = wc.tile([128, FC, 128], F8, tag="wf2")
                    nc.sync.dma_start(
                        out=wf2_ch[:],
                        in_=t["wf2"][i, :, :, dd * 128:(dd + 1) * 128])
                    ps2 = mmp.tile([128, 512], F32, tag="mm")
                    for j in range(FC // 2):
                        nc.tensor.matmul(
                            ps2[:], wf2_ch[:, 2 * j:2 * j + 2, :],
                            f_s[:, 2 * j:2 * j + 2,
                                sub * 512:(sub + 1) * 512],
                            start=(j == 0), stop=(j == FC // 2 - 1),
                            perf_mode=DR)
                    f2t = sqp.tile([128, 512], BF16, tag="f2t")
                    nc.scalar.activation(out=f2t[:], in_=ps2[:],
                                         func=AF.Identity,
                                         bias=consts["bf2_c"][:, i, dd:dd + 1],
                                         scale=CF2)
                    nc.vector.tensor_add(
                        out=res_t[:, dd, :], in0=f2t[:],
                        in1=h_s[:, dd, sub * 512:(sub + 1) * 512])
                    sq = sqp.tile([128, 512], F32R, tag="fsq")
                    nc.scalar.activation(out=sq[:],
                                         in_=res_t[:, dd, :].bitcast(F32),
                                         func=AF.Square)
                    nc.tensor.matmul(ps_s[:], ones_col_f[:, 0:1],
                                     res_t[:, dd, :],
                                     start=(dd == 0), stop=(dd == DC - 1))
                    nc.tensor.matmul(ps_q[:], ones_col_f[:, 0:1],
                                     sq[:],
                                     start=(dd == 0), stop=(dd == DC - 1))

                rs_row, mrs_row = _ln_rows(nc, lp, ps_s, ps_q, 512, 1.0 / D,
                                           eps1)
                brs = mmp.tile([128, 512], F32, tag="mm")
                nc.tensor.matmul(brs[:], ones_row_f[:],
                                 rs_row, start=True, stop=True)
                for dd in range(DC):
                    bms = mmp.tile([128, 512], F32, tag="mm")
                    nc.tensor.matmul(
                        bms[:],
                        consts["ln2_s_row"][:, i, dd * 128:(dd + 1) * 128],
                        mrs_row, start=True, stop=True)
                    u = sqp.tile([128, 512], F32, tag="fu")
                    nc.vector.scalar_tensor_tensor(
                        out=u[:], in0=res_t[:, dd, :].bitcast(F32),
                        scalar=consts["ln2_s_c"][:, i, dd:dd + 1], in1=brs[:],
                        op0=OP.mult, op1=OP.mult)
                    if SURVIVE[i] == 1.0:
                        nc.vector.scalar_tensor_tensor(
                            out=xT[:, dd, tok], in0=u[:],
                            scalar=consts["ln2_b_c"][:, i, dd:dd + 1],
                            in1=bms[:], op0=OP.add, op1=OP.subtract)
                    else:
                        v = sqp.tile([128, 512], F32, tag="fv")
                        nc.vector.scalar_tensor_tensor(
                            out=v[:], in0=u[:],
                            scalar=consts["ln2_b_c"][:, i, dd:dd + 1],
                            in1=bms[:], op0=OP.add, op1=OP.subtract)
                        nc.vector.scalar_tensor_tensor(
                            out=xT[:, dd, tok], in0=xT[:, dd, tok],
                            scalar=1.0 - SURVIVE[i], in1=v[:],
                            op0=OP.mult, op1=OP.add)
                    if i < NL - 1:
                        nc.gpsimd.tensor_copy(out=xT8[:, dd, tok],
                                              in_=xT[:, dd, tok])


def _head(nc, tc, t, consts, xT, ones_col_f, ones_row_f, eps1, mmp, stp, y_out):
    with (
        tc.tile_pool(name="head", bufs=1) as hp,
    ):
        wd1_s = hp.tile([128, 17, 128], BF16)
        nc.sync.dma_start(out=wd1_s[:], in_=t["wd1"][:])
        # pooled = concat([mean_L(x), max_L(x), other])  (feature-major)
        pool_sum = hp.tile([128, DC, BPC], F32)
        nc.vector.tensor_reduce(
            out=pool_sum[:], in_=xT.rearrange("p c (b l) -> p c b l", l=L),
            axis=mybir.AxisListType.X, op=OP.add)
        pool_max = hp.tile([128, DC, BPC], F32)
        nc.vector.tensor_reduce(
            out=pool_max[:], in_=xT.rearrange("p c (b l) -> p c b l", l=L),
            axis=mybir.AxisListType.X, op=OP.max)
        poolT = hp.tile([128, 17, BPC], BF16)
        nc.vector.tensor_scalar_mul(out=poolT[:, 0:DC, :], in0=pool_sum[:],
                                    scalar1=1.0 / L)
        nc.gpsimd.tensor_copy(out=poolT[:, DC:2 * DC, :], in_=pool_max[:])
        nc.gpsimd.tensor_copy(out=poolT[:, 16, :], in_=consts["otherT"][:])

        # y1 = relu(pooled @ Wd1 + bd1)   [128 dout, 16]
        ps1 = mmp.tile([128, 512], F32, tag="mm", name="hps")[:, 0:BPC]
        for c in range(17):
            nc.tensor.matmul(ps1, wd1_s[:, c, :], poolT[:, c, :],
                             start=(c == 0), stop=(c == 16))
        y1 = hp.tile([128, BPC], F32R)
        nc.scalar.activation(out=y1[:], in_=ps1, func=AF.Relu,
                             bias=consts["bd1_c"][:], scale=1.0)

        # LN3 over the 128 features (partition dim)
        sq3 = hp.tile([128, BPC], F32R)
        nc.scalar.activation(out=sq3[:], in_=y1[:].bitcast(F32),
                             func=AF.Square)
        ps_s = stp.tile([1, 512], F32, tag="s", name="hs")[:, 0:BPC]
        ps_q = stp.tile([1, 512], F32, tag="q", name="hq")[:, 0:BPC]
        nc.tensor.matmul(ps_s, ones_col_f[:, 0:1],
                         y1[:], start=True, stop=True)
        nc.tensor.matmul(ps_q, ones_col_f[:, 0:1],
                         sq3[:], start=True, stop=True)
        with tc.tile_pool(name="hln", bufs=1) as lp:
            rs_row, mrs_row = _ln_rows(nc, lp, ps_s, ps_q, BPC, 1.0 / 128,
                                       eps1)
            brs = mmp.tile([128, 512], F32, tag="mm", name="hps")[:, 0:BPC]
            nc.tensor.matmul(brs, ones_row_f[:],
                             rs_row, start=True, stop=True)
            bms = mmp.tile([128, 512], F32, tag="mm", name="hps")[:, 0:BPC]
            nc.tensor.matmul(bms, consts["ln3_s_row"][:],
                             mrs_row, start=True, stop=True)
            u3 = hp.tile([128, BPC], F32)
            nc.vector.scalar_tensor_tensor(
                out=u3[:], in0=y1[:].bitcast(F32), scalar=consts["ln3_s_c"][:],
                in1=brs, op0=OP.mult, op1=OP.mult)
            yln = hp.tile([128, BPC], BF16)
            nc.vector.scalar_tensor_tensor(
                out=yln[:], in0=u3[:], scalar=consts["ln3_b_c"][:], in1=bms,
                op0=OP.add, op1=OP.subtract)

        # y2 = relu(yln @ Wd2 + bd2); y = y2 @ Wout + bout
        ps2 = mmp.tile([128, 512], F32, tag="mm", name="hps")[:, 0:BPC]
        nc.tensor.matmul(ps2, consts["wd2"][:], yln[:], start=True, stop=True)
        y2 = hp.tile([128, BPC], BF16)
        nc.scalar.activation(out=y2[:], in_=ps2, func=AF.Relu,
                             bias=consts["bd2_c"][:], scale=1.0)
        psy = mmp.tile([128, 512], F32, tag="mm", name="hps1")[0:1, 0:BPC]
        nc.tensor.matmul(psy, consts["wout"][:], y2[:], start=True, stop=True)
        yfin = hp.tile([1, BPC], F32)
        nc.vector.tensor_tensor(
            out=yfin[:], in0=psy,
            in1=consts["bout_t"][:].to_broadcast([1, BPC]), op=OP.add)
        nc.sync.dma_start(out=y_out[:], in_=yfin[:])


# ---------------------------------------------------------------------------
# host side
# ---------------------------------------------------------------------------

def _bf(x):
    return np.ascontiguousarray(np.asarray(x, np.float32)).astype(
        ml_dtypes.bfloat16)


def _f8(x, scale):
    a = np.asarray(x, np.float32) * np.float32(scale)
    np.clip(a, -224.0, 224.0, out=a)
    return np.ascontiguousarray(a).astype(ml_dtypes.float8_e4m3)


def _f32(x):
    return np.ascontiguousarray(np.asarray(x, np.float32))


def _prep_shared(I):
    """Weight transforms shared by all cores."""
    sv = np.array(SURVIVE, np.float32)
    s = {}
    s["w_in"] = _bf(I["W_in"])
    s["b_in_c"] = _f32(I["b_in"].reshape(DC, 128).T)
    for nm, W in (("wq", I["Wq"]), ("wk", I["Wk"]), ("wv", I["Wv"]),
                  ("wg", I["Wg"]), ("wo", I["Wo"])):
        s[nm] = _f8(np.asarray(W, np.float32).reshape(NL, DC, 128, D)
                    .transpose(0, 2, 1, 3), SW)
    s["wf1"] = _f8(np.asarray(I["Wf1"], np.float32).reshape(NL, DC, 128, FF)
                   .transpose(0, 2, 1, 3), SW1)
    s["wfg"] = _f8(np.asarray(I["Wfg"], np.float32).reshape(NL, DC, 128, FF)
                   .transpose(0, 2, 1, 3), SW)
    s["wf2"] = _f8(np.asarray(I["Wf2"], np.float32).reshape(NL, FC, 128, D)
                   .transpose(0, 2, 1, 3), SW)

    def col(b, nch):
        return _f32(np.asarray(b).reshape(NL, nch, 128).transpose(2, 0, 1))

    s["bq_c"] = col(np.asarray(I["bq"], np.float32) * np.float32(SQ * QSCALE),
                    DC)
    s["bk_c"] = col(I["bk"], DC)
    s["bg_c"] = col(I["bg"], DC)
    s["bo_c"] = col(I["bo"], DC)
    s["bf2_c"] = col(I["bf2"], DC)
    s["bf1_c"] = col(np.asarray(I["bf1"], np.float32) * np.float32(SW1), FC)
    s["bfg_c"] = col(I["bfg"], FC)
    s["ln1_s_c"] = col(I["ln1_s"], DC)
    s["ln1_b_c"] = col(I["ln1_b"], DC)
    s["ln2_s_c"] = col(I["ln2_s"] * sv[:, None], DC)
    s["ln2_b_c"] = col(I["ln2_b"] * sv[:, None], DC)
    s["ln1_s_row"] = _f32(I["ln1_s"][None])
    s["ln2_s_row"] = _f32((I["ln2_s"] * sv[:, None])[None])
    s["bv_row"] = _bf(np.asarray(I["bv"], np.float32) * np.float32(SW))[None]
    s["wd1"] = _bf(np.concatenate(
        [np.asarray(I["Wd1"], np.float32),
         np.zeros((17 * 128 - I["Wd1"].shape[0], 128), np.float32)],
        axis=0).reshape(17, 128, 128).transpose(1, 0, 2))
    s["bd1_c"] = _f32(I["bd1"].reshape(128, 1))
    s["ln3_s_c"] = _f32(I["ln3_s"].reshape(128, 1))
    s["ln3_b_c"] = _f32(I["ln3_b"].reshape(128, 1))
    s["ln3_s_row"] = _f32(I["ln3_s"].reshape(1, 128))
    s["wd2"] = _bf(I["Wd2"])
    s["bd2_c"] = _f32(I["bd2"].reshape(128, 1))
    s["wout"] = _bf(I["Wout"])
    s["bout_t"] = _f32(I["bout"].reshape(1, 1))
    s["onesc"] = np.ones((128, 1), np.float32)
    s["onesr"] = np.ones((1, 128), np.float32)
    return s


def _prep_core(I, shared, c):
    m = dict(shared)
    cgm = np.asarray(I["cgm"], np.float32)
    m["cgmT"] = _bf(cgm[c * BPC:(c + 1) * BPC].reshape(NTOK, DFEAT).T)
    oth = np.asarray(I["other"], np.float32)[c * BPC:(c + 1) * BPC]  # [16,64]
    m["otherT"] = _bf(np.concatenate(
        [oth.T, np.zeros((128 - OTHER, BPC), np.float32)], axis=0))
    # exp of transposed+flipped rel-pos bias table, per-core column window
    rel = np.asarray(I["rel_emb"], np.float32)          # [NL, 255, 128]
    flippedT = rel[:, ::-1, :].transpose(0, 2, 1)       # [NL, 128, 255]
    lo = 112 - 16 * c
    tab = flippedT[:, :, lo:lo + 143]                   # [NL, 128, 143]
    m["exptab"] = _bf(np.exp(tab).transpose(1, 0, 2))   # [128, NL, 143]
    return m


def kernel(**inputs) -> np.ndarray:
    if "nc" not in _cached:
        _cached["nc"] = _build_nc()
    nc = _cached["nc"]
    shared = _prep_shared(inputs)
    in_maps = [_prep_core(inputs, shared, c) for c in range(NCORES)]
    res = run_bass_kernel_spmd(nc, in_maps, core_ids=list(range(NCORES)))
    y = np.concatenate([res.results[c]["y"].reshape(BPC)
                        for c in range(NCORES)])
    return y.reshape(B, 1).astype(np.float32)


# revision 8
# speedup vs baseline: 1180.0640x; 1180.0640x over previous
"""Trainium2 Bass kernel for nn_AttentionModel_63737314672806.

Sharding: data-parallel over batch (B=128) across 8 NeuronCores; each core
processes 16 batch elements (2048 tokens) through the full model. Weights are
replicated (broadcast) to every core. No collectives.

Device layout: activations are kept feature-major ("transposed"):
  xT[p, c, t] = x[token t, feature c*128+p]   (SBUF tile [128, 8, 2048])
so every dense layer is psum[dout, tok] = sum_kc matmul(lhsT=W[kc, dout_chunk],
rhs=xT[kc, tok_tile]) and the output is feature-major again (no transposes).
LayerNorm reductions over features (partition dim) use ones-vector matmuls;
per-token scalars are broadcast along partitions with rank-1 matmuls.
Softmax is computed in the transposed attention layout awT[k, q] (which falls
out of matmul(lhsT=kT, rhs=qT)) so no transposes are needed in attention
either; the relative-position bias is applied as exp(logit)*exp(bias) with a
host-precomputed exp-table.

Precision: the large projections/FFN matmuls run in fp8 e4m3 with
perf_mode=DoubleRow (2 fp8 weights per PE cell -> 2x throughput). Weights are
host-quantized with power-of-2 per-matrix scales (SW=1024, SW1=32 for Wf1);
descales fold into existing epilogue constants. Residual, LayerNorm and
pooling paths stay bf16/fp32: fp8 copies of x and h are produced on the
(otherwise idle) gpsimd engine purely as matmul inputs. Attention q/k/v/aw
are fp8 (q pre-scaled by 32 so layer-0 values clear the fp8 subnormal range;
the 1/32 folds into the exp's scale operand). Softmax/LN statistics are fp32.

Measured (numpy emulation of this exact quantization): rel-l2 4.2e-3 vs the
fp32 reference.
"""

import math

import numpy as np
import ml_dtypes

import concourse.bass as bass
import concourse.bacc as bacc
import concourse.mybir as mybir
import concourse.tile as tile
from concourse.bass_utils import run_bass_kernel_spmd

BF16 = mybir.dt.bfloat16
F8 = mybir.dt.float8e4
F32 = mybir.dt.float32
F32R = mybir.dt.float32r
AF = mybir.ActivationFunctionType
OP = mybir.AluOpType
DR = mybir.MatmulPerfMode.DoubleRow

NCORES = 8
B = 128
L = 128
DFEAT = 32
H = 8
DK = 128
D = 1024  # = H * DK
FF = 4096
NL = 2
MAXPOS = 128
OTHER = 64
EPS = 1e-6

BPC = B // NCORES       # 16 batches per core
NTOK = BPC * L          # 2048 tokens per core
NG = 4                  # batch groups per core (4 batches / 512 tokens each)
GB = BPC // NG          # batches per group = 4
GT = GB * L             # tokens per group = 512
DC = D // 128           # 8 feature chunks
FC = FF // 128          # 32 ff chunks
QSCALE = 1.0 / math.sqrt(float(DK))
SURVIVE = [1.0, 0.5]    # jnp.linspace(1.0, 0.5, 2)

# fp8 scale plan (all powers of 2; folded into epilogue constants)
SW = 1024.0             # weight scale: wq wk wv wg wo wfg wf2
SW1 = 32.0              # weight scale for wf1 (its descale rides on f8)
SQ = 32.0               # q stored as 32*q
SV = 32.0               # v stored as 32*v
SAO = 32.0              # ao stored as 32*ao  (SV==SAO makes rb == 1/sm)
SF = SW1                # f stored as 32*f

CQ = SQ * QSCALE / SW
CK = 1.0 / SW
CV = SV / SW
CO = 1.0 / (SW * SAO)
CG = 1.0 / SW
CFG = 1.0 / SW
CF2 = 1.0 / (SW * SF)
EXPS = 1.0 / SQ

_cached = {}


def _build_nc():
    nc = bacc.Bacc("TRN2", target_bir_lowering=False, debug=False,
                   num_devices=NCORES)

    def din(name, shape, dtype):
        return nc.dram_tensor(name, list(shape), dtype, kind="ExternalInput")

    t = {}
    t["cgmT"] = din("cgmT", [DFEAT, NTOK], BF16)
    t["w_in"] = din("w_in", [DFEAT, D], BF16)
    t["b_in_c"] = din("b_in_c", [128, DC], F32)
    for w in ("wq", "wk", "wv"):
        t[w] = din(w, [NL, 128, DC, D], F8)
    # wo+wg fused, chunked on the output dim so each DMA is contiguous
    t["wog"] = din("wog", [NL, DC, 128, 2, DC, 128], F8)
    # wf1+wfg fused, 4 ff-chunks (512 cols) per load
    t["wf12"] = din("wf12", [NL, FC // 4, 128, 2, DC, 512], F8)
    # wf2 in 2-output-chunk loads
    t["wf2"] = din("wf2", [NL, DC // 2, 128, 2, FC, 128], F8)
    for bn in ("bq_c", "bk_c", "bg_c", "bo_c", "bf2_c",
               "ln1_s_c", "ln1_b_c", "ln2_s_c", "ln2_b_c"):
        t[bn] = din(bn, [128, NL, DC], F32)
    t["bf1_c"] = din("bf1_c", [128, NL, FC], F32)
    t["bfg_c"] = din("bfg_c", [128, NL, FC], F32)
    t["ln1_s_row"] = din("ln1_s_row", [1, NL, D], F32R)
    t["ln2_s_row"] = din("ln2_s_row", [1, NL, D], F32R)
    t["bv_row"] = din("bv_row", [1, NL, D], BF16)
    t["exptab"] = din("exptab", [128, NL, 143], BF16)
    t["wd1"] = din("wd1", [128, 17, 128], BF16)
    t["bd1_c"] = din("bd1_c", [128, 1], F32)
    t["ln3_s_c"] = din("ln3_s_c", [128, 1], F32)
    t["ln3_b_c"] = din("ln3_b_c", [128, 1], F32)
    t["ln3_s_row"] = din("ln3_s_row", [1, 128], F32R)
    t["wd2"] = din("wd2", [128, 128], BF16)
    t["bd2_c"] = din("bd2_c", [128, 1], F32)
    t["wout"] = din("wout", [128, 1], BF16)
    t["bout_t"] = din("bout_t", [1, 1], F32)
    t["otherT"] = din("otherT", [128, BPC], BF16)
    t["onesc"] = din("onesc", [128, 1], F32R)
    t["onesr"] = din("onesr", [1, 128], F32R)
    y_out = nc.dram_tensor("y", [1, BPC], F32, kind="ExternalOutput")

    with tile.TileContext(nc, pool_alloc_mode="queue") as tc:
        _emit(nc, tc, t, y_out)
    nc.compile()
    return nc


def _emit(nc, tc, t, y_out):
    with (
        tc.tile_pool(name="persist", bufs=1) as pp,
        tc.tile_pool(name="dramp", bufs=1, space="DRAM") as dp,
        tc.tile_pool(name="mm_psum", bufs=6, space="PSUM") as mmp,
        tc.tile_pool(name="stat_psum", bufs=1, space="PSUM") as stp,
    ):
        # ---- persistent SBUF state ----
        xT = pp.tile([128, DC, NTOK], BF16)
        xT8 = pp.tile([128, DC, NTOK], F8)
        h8 = pp.tile([128, DC, NTOK], F8)
        ones_col_bf = pp.tile([128, 1], BF16)
        nc.vector.memset(ones_col_bf, 1.0)
        ones_col_8 = pp.tile([128, 1], F8)
        nc.vector.memset(ones_col_8, 1.0)
        ones_row_bf = pp.tile([1, 128], BF16)
        nc.vector.memset(ones_row_bf, 1.0)
        ones_col_f = pp.tile([128, 1], F32R)
        nc.sync.dma_start(out=ones_col_f[:], in_=t["onesc"][:])
        ones_row_f = pp.tile([1, 128], F32R)
        nc.sync.dma_start(out=ones_row_f[:], in_=t["onesr"][:])
        eps1 = pp.tile([1, 1], F32)
        nc.vector.memset(eps1, EPS)

        # input projection operands first: the first matmuls need them
        early = {}
        for name in ("cgmT", "w_in"):
            ap = t[name]
            tl = pp.tile(list(ap.shape), ap.dtype, name=f"e_{name}")
            nc.sync.dma_start(out=tl[:], in_=ap[:])
            early[name] = tl

        # small constants from DRAM
        consts = {}
        for name in ("b_in_c", "bq_c", "bk_c", "bg_c", "bo_c", "bf2_c",
                     "ln1_s_c", "ln1_b_c", "ln2_s_c", "ln2_b_c",
                     "bf1_c", "bfg_c", "ln1_s_row", "ln2_s_row", "bv_row",
                     "exptab", "bd1_c", "ln3_s_c", "ln3_b_c",
                     "ln3_s_row", "wd2", "bd2_c", "wout", "bout_t", "otherT"):
            ap = t[name]
            tl = pp.tile(list(ap.shape), ap.dtype, name=f"c_{name}")
            nc.sync.dma_start(out=tl[:], in_=ap[:])
            consts[name] = tl

        # ---- input projection: xT = cgm @ W_in + b_in ----
        cgmT_s = early["cgmT"]
        w_in_s = early["w_in"]
        for dd in range(DC):
            for g in range(NG):
                ps = mmp.tile([128, GT], F32, tag="mm")
                nc.tensor.matmul(ps[:], w_in_s[:, dd * 128:(dd + 1) * 128],
                                 cgmT_s[:, g * GT:(g + 1) * GT],
                                 start=True, stop=True)
                nc.vector.tensor_scalar_add(
                    out=xT[:, dd, g * GT:(g + 1) * GT], in0=ps[:],
                    scalar1=consts["b_in_c"][:, dd:dd + 1])
                nc.gpsimd.tensor_copy(
                    out=xT8[:, dd, g * GT:(g + 1) * GT],
                    in_=xT[:, dd, g * GT:(g + 1) * GT])

        h_dram = dp.tile([128, DC, NTOK], BF16)

        # ---- transformer layers ----
        for i in range(NL):
            _layer_attn(nc, tc, t, consts, i, xT, xT8, h8, h_dram,
                        ones_col_bf, ones_col_8, ones_row_bf, ones_col_f,
                        ones_row_f, eps1, mmp, stp)
            _layer_ffn(nc, tc, t, consts, i, xT, xT8, h8, h_dram,
                       ones_col_f, ones_row_f, eps1, mmp, stp)

        # ---- head ----
        _head(nc, tc, t, consts, xT, ones_col_f, ones_row_f, eps1, mmp, stp,
              y_out)


def _ln_rows(nc, lp, psum_s, psum_q, n, inv_d, eps1):
    """From psum_s=sum(x) and psum_q=sum(x^2) over features ([1, n] each),
    compute rs_row = 1/sqrt(var+eps) and mrs_row = mean*rs ([1, n] f32)."""
    m_row = lp.tile([1, 512], F32R, tag="m", name="m_row")[:, :n]
    with nc.allow_low_precision(reason="fp32r LN stats within tolerance"):
        nc.vector.tensor_scalar_mul(out=m_row, in0=psum_s[:], scalar1=inv_d)
    m_f = m_row.bitcast(F32)
    ex2 = lp.tile([1, 512], F32, tag="e", name="ex2")[:, :n]
    nc.vector.tensor_scalar_mul(out=ex2, in0=psum_q[:], scalar1=inv_d)
    var = lp.tile([1, 512], F32, tag="v", name="var")[:, :n]
    # var = ex2 - m*m
    nc.vector.scalar_tensor_tensor(out=var, in0=m_f, scalar=-1.0, in1=m_f,
                                   op0=OP.mult, op1=OP.mult)
    nc.vector.tensor_add(out=var, in0=var, in1=ex2)
    nc.scalar.activation(out=var, in_=var, func=AF.Sqrt, bias=eps1[:],
                         scale=1.0)
    rs_row = lp.tile([1, 512], F32R, tag="r", name="rs_row")[:, :n]
    with nc.allow_low_precision(reason="fp32r LN stats within tolerance"):
        nc.vector.reciprocal(out=rs_row, in_=var)
        nc.vector.tensor_mul(out=m_row, in0=m_f, in1=rs_row.bitcast(F32))
    return rs_row, m_row


def _layer_attn(nc, tc, t, consts, i, xT, xT8, h8, h_dram, ones_col_bf,
                ones_col_8, ones_row_bf, ones_col_f, ones_row_f, eps1, mmp,
                stp):
    with (
        tc.tile_pool(name="wqk", bufs=1) as wp,
        tc.tile_pool(name="grp", bufs=2) as gp,
        tc.tile_pool(name="wch", bufs=2) as wc,
        tc.tile_pool(name="wvch", bufs=2) as wvc,
        tc.tile_pool(name="aog", bufs=1) as aop,
        tc.tile_pool(name="att", bufs=2) as at,
        tc.tile_pool(name="res", bufs=1) as rp,
        tc.tile_pool(name="sq", bufs=2) as sqp,
        tc.tile_pool(name="upool", bufs=1) as up,
        tc.tile_pool(name="lnsm", bufs=1) as lp,
        tc.tile_pool(name="hst", bufs=1) as hstp,
    ):
        wq_s = wp.tile([128, DC, D], F8)
        nc.sync.dma_start(out=wq_s[:], in_=t["wq"][i])
        wk_s = wp.tile([128, DC, D], F8)
        nc.sync.dma_start(out=wk_s[:], in_=t["wk"][i])

        for g in range(NG):
            tok = slice(g * GT, (g + 1) * GT)
            # --- Q/K projections (feature-major; q carries SQ*QSCALE) ---
            qT_g = gp.tile([128, DC, GT], F8, tag="q")
            kT_g = gp.tile([128, DC, GT], F8, tag="k")
            for dd in range(DC):
                psq = mmp.tile([128, GT], F32, tag="mm")
                psk = mmp.tile([128, GT], F32, tag="mm")
                for j in range(DC // 2):
                    nc.tensor.matmul(
                        psq[:], wq_s[:, 2 * j:2 * j + 2,
                                     dd * 128:(dd + 1) * 128],
                        xT8[:, 2 * j:2 * j + 2, tok],
                        start=(j == 0), stop=(j == DC // 2 - 1), perf_mode=DR)
                for j in range(DC // 2):
                    nc.tensor.matmul(
                        psk[:], wk_s[:, 2 * j:2 * j + 2,
                                     dd * 128:(dd + 1) * 128],
                        xT8[:, 2 * j:2 * j + 2, tok],
                        start=(j == 0), stop=(j == DC // 2 - 1), perf_mode=DR)
                nc.vector.tensor_scalar(
                    out=qT_g[:, dd, :], in0=psq[:], scalar1=CQ,
                    scalar2=consts["bq_c"][:, i, dd:dd + 1],
                    op0=OP.mult, op1=OP.add)
                nc.vector.tensor_scalar(
                    out=kT_g[:, dd, :], in0=psk[:], scalar1=CK,
                    scalar2=consts["bk_c"][:, i, dd:dd + 1],
                    op0=OP.mult, op1=OP.add)

            # --- V projection (token-major, stored as SV*v) ---
            v_g = gp.tile([128, GB, D], F8, tag="v")
            for cc in range(2):
                wv_ch = wvc.tile([128, DC, 512], F8, tag="wv")
                nc.gpsimd.dma_start(out=wv_ch[:],
                                  in_=t["wv"][i, :, :, cc * 512:(cc + 1) * 512])
                for jj in range(GB):
                    btok = slice((g * GB + jj) * L, (g * GB + jj + 1) * L)
                    psv = mmp.tile([128, 512], F32, tag="mm")
                    for j in range(DC // 2):
                        nc.tensor.matmul(psv[:],
                                         xT8[:, 2 * j:2 * j + 2, btok],
                                         wv_ch[:, 2 * j:2 * j + 2, :],
                                         start=(j == 0), stop=False,
                                         perf_mode=DR)
                    nc.tensor.matmul(psv[:], ones_row_bf[:],
                                     consts["bv_row"][:, i,
                                                      cc * 512:(cc + 1) * 512],
                                     start=False, stop=True)
                    nc.scalar.activation(
                        out=v_g[:, jj, cc * 512:(cc + 1) * 512], in_=psv[:],
                        func=AF.Copy, scale=CV)

            # --- attention per (batch, head) ---
            ao_g = aop.tile([128, DC, GT], F8, tag="ao")
            for jj in range(GB):
                b_local = g * GB + jj
                jtok = slice(jj * L, (jj + 1) * L)
                etab = consts["exptab"][:, i, 15 - b_local:143 - b_local]
                for hh in range(H):
                    pa = mmp.tile([128, 512], F32, tag="mm", name="pa")
                    aw = pa[:, 0:128]
                    ao = pa[:, 256:384]
                    sm = pa[0:1, 384:512]
                    nc.tensor.matmul(aw, kT_g[:, hh, jtok], qT_g[:, hh, jtok],
                                     start=True, stop=True)
                    awe = at.tile([128, 128], BF16, tag="awe")
                    nc.scalar.activation(out=awe[:], in_=aw, func=AF.Exp,
                                         scale=EXPS)
                    awe2 = at.tile([128, 128], F8, tag="awe2")
                    nc.vector.tensor_mul(out=awe2[:], in0=awe[:], in1=etab)
                    nc.tensor.matmul(sm, ones_col_8[:, 0:1], awe2[:, 0:128],
                                     start=True, stop=True)
                    rc = at.tile([1, 128], BF16, tag="rc")
                    with nc.allow_low_precision(
                            reason="bf16 softmax normalizer is within tolerance"):
                        nc.vector.reciprocal(out=rc[:], in_=sm)
                    rb = at.tile([128, 128], BF16, tag="rb")
                    nc.gpsimd.partition_broadcast(rb[:], rc[:])
                    nc.tensor.matmul(ao, v_g[:, jj, hh * 128:(hh + 1) * 128],
                                     awe2[:], start=True, stop=True)
                    nc.vector.tensor_mul(out=ao_g[:, hh, jtok], in0=ao,
                                         in1=rb[:])

            # --- o-proj + gate + residual + LN1 stats ---
            res_t = rp.tile([128, DC, GT], BF16, tag="res")
            ps_s = stp.tile([1, GT], F32, tag="s")
            ps_q = stp.tile([1, GT], F32, tag="q")
            for dd in range(DC):
                wo_ch = wc.tile([128, DC, 128], F8, tag="wo")
                nc.sync.dma_start(out=wo_ch[:],
                                  in_=t["wo"][i, :, :, dd * 128:(dd + 1) * 128])
                wg_ch = wc.tile([128, DC, 128], F8, tag="wg")
                nc.sync.dma_start(out=wg_ch[:],
                                  in_=t["wg"][i, :, :, dd * 128:(dd + 1) * 128])
                pso = mmp.tile([128, GT], F32, tag="mm")
                psg = mmp.tile([128, GT], F32, tag="mm")
                for j in range(DC // 2):
                    nc.tensor.matmul(pso[:], wo_ch[:, 2 * j:2 * j + 2, :],
                                     ao_g[:, 2 * j:2 * j + 2, :],
                                     start=(j == 0), stop=(j == DC // 2 - 1),
                                     perf_mode=DR)
                for j in range(DC // 2):
                    nc.tensor.matmul(psg[:], wg_ch[:, 2 * j:2 * j + 2, :],
                                     xT8[:, 2 * j:2 * j + 2, tok],
                                     start=(j == 0), stop=(j == DC // 2 - 1),
                                     perf_mode=DR)
                sig = sqp.tile([128, GT], BF16, tag="sig")
                nc.scalar.activation(out=sig[:], in_=psg[:], func=AF.Sigmoid,
                                     bias=consts["bg_c"][:, i, dd:dd + 1],
                                     scale=CG)
                ot = sqp.tile([128, GT], BF16, tag="ot")
                nc.scalar.activation(out=ot[:], in_=pso[:],
                                     func=AF.Identity,
                                     bias=consts["bo_c"][:, i, dd:dd + 1],
                                     scale=CO)
                # res = x + sig * (o + bo)
                nc.vector.tensor_mul(out=res_t[:, dd, :], in0=ot[:],
                                     in1=sig[:])
                nc.vector.tensor_add(out=res_t[:, dd, :],
                                     in0=res_t[:, dd, :], in1=xT[:, dd, tok])
                sq = sqp.tile([128, GT], BF16, tag="sq")
                nc.scalar.activation(out=sq[:], in_=res_t[:, dd, :],
                                     func=AF.Square)
                nc.tensor.matmul(ps_s[:], ones_col_bf[:, 0:1],
                                 res_t[:, dd, :],
                                 start=(dd == 0), stop=(dd == DC - 1))
                nc.tensor.matmul(ps_q[:], ones_col_bf[:, 0:1],
                                 sq[:],
                                 start=(dd == 0), stop=(dd == DC - 1))

            # --- LN1 apply -> h (bf16 to DRAM for the residual; fp8 copy
            # in SBUF for the FFN matmuls) ---
            rs_row, mrs_row = _ln_rows(nc, lp, ps_s, ps_q, GT, 1.0 / D, eps1)
            brs = mmp.tile([128, GT], F32, tag="mm")
            nc.tensor.matmul(brs[:], ones_row_f[:],
                             rs_row, start=True, stop=True)
            h_g = hstp.tile([128, DC, GT], BF16, tag="hg")
            for dd in range(DC):
                bms = mmp.tile([128, GT], F32, tag="mm")
                nc.tensor.matmul(
                    bms[:],
                    consts["ln1_s_row"][:, i, dd * 128:(dd + 1) * 128],
                    mrs_row, start=True, stop=True)
                u = up.tile([128, GT], F32, tag="u")
                nc.vector.scalar_tensor_tensor(
                    out=u[:], in0=res_t[:, dd, :],
                    scalar=consts["ln1_s_c"][:, i, dd:dd + 1], in1=brs[:],
                    op0=OP.mult, op1=OP.mult)
                nc.vector.scalar_tensor_tensor(
                    out=h_g[:, dd, :], in0=u[:],
                    scalar=consts["ln1_b_c"][:, i, dd:dd + 1], in1=bms[:],
                    op0=OP.add, op1=OP.subtract)
            nc.gpsimd.tensor_copy(out=h8[:, :, tok], in_=h_g[:])
            nc.sync.dma_start(out=h_dram[:, :, tok], in_=h_g[:])


def _layer_ffn(nc, tc, t, consts, i, xT, xT8, h8, h_dram, ones_col_f,
               ones_row_f, eps1, mmp, stp):
    with (
        tc.tile_pool(name="fbuf", bufs=1) as fp,
        tc.tile_pool(name="fwch", bufs=2) as wc,
        tc.tile_pool(name="fres", bufs=1) as rp,
        tc.tile_pool(name="fsq", bufs=2) as sqp,
        tc.tile_pool(name="flnsm", bufs=1) as lp,
        tc.tile_pool(name="hsup", bufs=1) as hp2,
    ):
        for sup in range(2):
            f_s = fp.tile([128, FC, 1024], F8, tag="f")
            stok0 = sup * 1024
            h_s = hp2.tile([128, DC, 1024], BF16, tag="hs")
            nc.sync.dma_start(out=h_s[:],
                              in_=h_dram[:, :, stok0:stok0 + 1024])
            # --- f = SF * (h@Wf1 + bf1) * sigmoid(h@Wfg + bfg) ---
            for fc in range(FC):
                wf1_ch = wc.tile([128, DC, 128], F8, tag="wf1")
                nc.gpsimd.dma_start(out=wf1_ch[:],
                                  in_=t["wf1"][i, :, :, fc * 128:(fc + 1) * 128])
                wfg_ch = wc.tile([128, DC, 128], F8, tag="wfg")
                nc.gpsimd.dma_start(out=wfg_ch[:],
                                  in_=t["wfg"][i, :, :, fc * 128:(fc + 1) * 128])
                for sub in range(2):
                    tok8 = slice(stok0 + sub * 512, stok0 + (sub + 1) * 512)
                    ps1 = mmp.tile([128, 512], F32, tag="mm")
                    psg = mmp.tile([128, 512], F32, tag="mm")
                    for j in range(DC // 2):
                        nc.tensor.matmul(ps1[:], wf1_ch[:, 2 * j:2 * j + 2, :],
                                         h8[:, 2 * j:2 * j + 2, tok8],
                                         start=(j == 0),
                                         stop=(j == DC // 2 - 1), perf_mode=DR)
                    for j in range(DC // 2):
                        nc.tensor.matmul(psg[:], wfg_ch[:, 2 * j:2 * j + 2, :],
                                         h8[:, 2 * j:2 * j + 2, tok8],
                                         start=(j == 0),
                                         stop=(j == DC // 2 - 1), perf_mode=DR)
                    sig = sqp.tile([128, 512], BF16, tag="fsig")
                    nc.scalar.activation(out=sig[:], in_=psg[:],
                                         func=AF.Sigmoid,
                                         bias=consts["bfg_c"][:, i, fc:fc + 1],
                                         scale=CFG)
                    nc.vector.scalar_tensor_tensor(
                        out=f_s[:, fc, sub * 512:(sub + 1) * 512], in0=ps1[:],
                        scalar=consts["bf1_c"][:, i, fc:fc + 1], in1=sig[:],
                        op0=OP.add, op1=OP.mult)

            # --- f @ Wf2 + bf2 + h, then LN2 + stochastic-depth residual ---
            for sub in range(2):
                tok = slice(stok0 + sub * 512, stok0 + (sub + 1) * 512)
                res_t = rp.tile([128, DC, 512], F32R, tag="res2")
                ps_s = stp.tile([1, 512], F32, tag="s")
                ps_q = stp.tile([1, 512], F32, tag="q")
                for dd in range(DC):
                    wf2_ch = wc.tile([128, FC, 128], F8, tag="wf2")
                    nc.sync.dma_start(
                        out=wf2_ch[:],
                        in_=t["wf2"][i, :, :, dd * 128:(dd + 1) * 128])
                    ps2 = mmp.tile([128, 512], F32, tag="mm")
                    for j in range(FC // 2):
                        nc.tensor.matmul(
                            ps2[:], wf2_ch[:, 2 * j:2 * j + 2, :],
                            f_s[:, 2 * j:2 * j + 2,
                                sub * 512:(sub + 1) * 512],
                            start=(j == 0), stop=(j == FC // 2 - 1),
                            perf_mode=DR)
                    f2t = sqp.tile([128, 512], BF16, tag="f2t")
                    nc.scalar.activation(out=f2t[:], in_=ps2[:],
                                         func=AF.Identity,
                                         bias=consts["bf2_c"][:, i, dd:dd + 1],
                                         scale=CF2)
                    nc.vector.tensor_add(
                        out=res_t[:, dd, :], in0=f2t[:],
                        in1=h_s[:, dd, sub * 512:(sub + 1) * 512])
                    sq = sqp.tile([128, 512], F32R, tag="fsq")
                    nc.scalar.activation(out=sq[:],
                                         in_=res_t[:, dd, :].bitcast(F32),
                                         func=AF.Square)
                    nc.tensor.matmul(ps_s[:], ones_col_f[:, 0:1],
                                     res_t[:, dd, :],
                                     start=(dd == 0), stop=(dd == DC - 1))
                    nc.tensor.matmul(ps_q[:], ones_col_f[:, 0:1],
                                     sq[:],
                                     start=(dd == 0), stop=(dd == DC - 1))

                rs_row, mrs_row = _ln_rows(nc, lp, ps_s, ps_q, 512, 1.0 / D,
                                           eps1)
                brs = mmp.tile([128, 512], F32, tag="mm")
                nc.tensor.matmul(brs[:], ones_row_f[:],
                                 rs_row, start=True, stop=True)
                for dd in range(DC):
                    bms = mmp.tile([128, 512], F32, tag="mm")
                    nc.tensor.matmul(
                        bms[:],
                        consts["ln2_s_row"][:, i, dd * 128:(dd + 1) * 128],
                        mrs_row, start=True, stop=True)
                    u = sqp.tile([128, 512], F32, tag="fu")
                    nc.vector.scalar_tensor_tensor(
                        out=u[:], in0=res_t[:, dd, :].bitcast(F32),
                        scalar=consts["ln2_s_c"][:, i, dd:dd + 1], in1=brs[:],
                        op0=OP.mult, op1=OP.mult)
                    if SURVIVE[i] == 1.0:
                        nc.vector.scalar_tensor_tensor(
                            out=xT[:, dd, tok], in0=u[:],
                            scalar=consts["ln2_b_c"][:, i, dd:dd + 1],
                            in1=bms[:], op0=OP.add, op1=OP.subtract)
                    else:
                        v = sqp.tile([128, 512], F32, tag="fv")
                        nc.vector.scalar_tensor_tensor(
                            out=v[:], in0=u[:],
                            scalar=consts["ln2_b_c"][:, i, dd:dd + 1],
                            in1=bms[:], op0=OP.add, op1=OP.subtract)
                        nc.vector.scalar_tensor_tensor(
                            out=xT[:, dd, tok], in0=xT[:, dd, tok],
                            scalar=1.0 - SURVIVE[i], in1=v[:],
                            op0=OP.mult, op1=OP.add)
                    if i < NL - 1:
                        nc.gpsimd.tensor_copy(out=xT8[:, dd, tok],
                                              in_=xT[:, dd, tok])


def _head(nc, tc, t, consts, xT, ones_col_f, ones_row_f, eps1, mmp, stp, y_out):
    with (
        tc.tile_pool(name="head", bufs=1) as hp,
    ):
        wd1_s = hp.tile([128, 17, 128], BF16)
        nc.sync.dma_start(out=wd1_s[:], in_=t["wd1"][:])
        # pooled = concat([mean_L(x), max_L(x), other])  (feature-major)
        pool_sum = hp.tile([128, DC, BPC], F32)
        nc.vector.tensor_reduce(
            out=pool_sum[:], in_=xT.rearrange("p c (b l) -> p c b l", l=L),
            axis=mybir.AxisListType.X, op=OP.add)
        pool_max = hp.tile([128, DC, BPC], F32)
        nc.vector.tensor_reduce(
            out=pool_max[:], in_=xT.rearrange("p c (b l) -> p c b l", l=L),
            axis=mybir.AxisListType.X, op=OP.max)
        poolT = hp.tile([128, 17, BPC], BF16)
        nc.vector.tensor_scalar_mul(out=poolT[:, 0:DC, :], in0=pool_sum[:],
                                    scalar1=1.0 / L)
        nc.gpsimd.tensor_copy(out=poolT[:, DC:2 * DC, :], in_=pool_max[:])
        nc.gpsimd.tensor_copy(out=poolT[:, 16, :], in_=consts["otherT"][:])

        # y1 = relu(pooled @ Wd1 + bd1)   [128 dout, 16]
        ps1 = mmp.tile([128, 512], F32, tag="mm", name="hps")[:, 0:BPC]
        for c in range(17):
            nc.tensor.matmul(ps1, wd1_s[:, c, :], poolT[:, c, :],
                             start=(c == 0), stop=(c == 16))
        y1 = hp.tile([128, BPC], F32R)
        nc.scalar.activation(out=y1[:], in_=ps1, func=AF.Relu,
                             bias=consts["bd1_c"][:], scale=1.0)

        # LN3 over the 128 features (partition dim)
        sq3 = hp.tile([128, BPC], F32R)
        nc.scalar.activation(out=sq3[:], in_=y1[:].bitcast(F32),
                             func=AF.Square)
        ps_s = stp.tile([1, 512], F32, tag="s", name="hs")[:, 0:BPC]
        ps_q = stp.tile([1, 512], F32, tag="q", name="hq")[:, 0:BPC]
        nc.tensor.matmul(ps_s, ones_col_f[:, 0:1],
                         y1[:], start=True, stop=True)
        nc.tensor.matmul(ps_q, ones_col_f[:, 0:1],
                         sq3[:], start=True, stop=True)
        with tc.tile_pool(name="hln", bufs=1) as lp:
            rs_row, mrs_row = _ln_rows(nc, lp, ps_s, ps_q, BPC, 1.0 / 128,
                                       eps1)
            brs = mmp.tile([128, 512], F32, tag="mm", name="hps")[:, 0:BPC]
            nc.tensor.matmul(brs, ones_row_f[:],
                             rs_row, start=True, stop=True)
            bms = mmp.tile([128, 512], F32, tag="mm", name="hps")[:, 0:BPC]
            nc.tensor.matmul(bms, consts["ln3_s_row"][:],
                             mrs_row, start=True, stop=True)
            u3 = hp.tile([128, BPC], F32)
            nc.vector.scalar_tensor_tensor(
                out=u3[:], in0=y1[:].bitcast(F32), scalar=consts["ln3_s_c"][:],
                in1=brs, op0=OP.mult, op1=OP.mult)
            yln = hp.tile([128, BPC], BF16)
            nc.vector.scalar_tensor_tensor(
                out=yln[:], in0=u3[:], scalar=consts["ln3_b_c"][:], in1=bms,
                op0=OP.add, op1=OP.subtract)

        # y2 = relu(yln @ Wd2 + bd2); y = y2 @ Wout + bout
        ps2 = mmp.tile([128, 512], F32, tag="mm", name="hps")[:, 0:BPC]
        nc.tensor.matmul(ps2, consts["wd2"][:], yln[:], start=True, stop=True)
        y2 = hp.tile([128, BPC], BF16)
        nc.scalar.activation(out=y2[:], in_=ps2, func=AF.Relu,
                             bias=consts["bd2_c"][:], scale=1.0)
        psy = mmp.tile([128, 512], F32, tag="mm", name="hps1")[0:1, 0:BPC]
        nc.tensor.matmul(psy, consts["wout"][:], y2[:], start=True, stop=True)
        yfin = hp.tile([1, BPC], F32)
        nc.vector.tensor_tensor(
            out=yfin[:], in0=psy,
            in1=consts["bout_t"][:].to_broadcast([1, BPC]), op=OP.add)
        nc.sync.dma_start(out=y_out[:], in_=yfin[:])


# ---------------------------------------------------------------------------
# host side
# ---------------------------------------------------------------------------

def _bf(x):
    return np.ascontiguousarray(np.asarray(x, np.float32)).astype(
        ml_dtypes.bfloat16)


def _f8(x, scale):
    a = np.asarray(x, np.float32) * np.float32(scale)
    np.clip(a, -224.0, 224.0, out=a)
    return np.ascontiguousarray(a).astype(ml_dtypes.float8_e4m3)


def _f32(x):
    return np.ascontiguousarray(np.asarray(x, np.float32))


def _prep_shared(I):
    """Weight transforms shared by all cores."""
    sv = np.array(SURVIVE, np.float32)
    s = {}
    s["w_in"] = _bf(I["W_in"])
    s["b_in_c"] = _f32(I["b_in"].reshape(DC, 128).T)
    for nm, W in (("wq", I["Wq"]), ("wk", I["Wk"]), ("wv", I["Wv"]),
                  ("wg", I["Wg"]), ("wo", I["Wo"])):
        s[nm] = _f8(np.asarray(W, np.float32).reshape(NL, DC, 128, D)
                    .transpose(0, 2, 1, 3), SW)
    s["wf1"] = _f8(np.asarray(I["Wf1"], np.float32).reshape(NL, DC, 128, FF)
                   .transpose(0, 2, 1, 3), SW1)
    s["wfg"] = _f8(np.asarray(I["Wfg"], np.float32).reshape(NL, DC, 128, FF)
                   .transpose(0, 2, 1, 3), SW)
    s["wf2"] = _f8(np.asarray(I["Wf2"], np.float32).reshape(NL, FC, 128, D)
                   .transpose(0, 2, 1, 3), SW)

    def col(b, nch):
        return _f32(np.asarray(b).reshape(NL, nch, 128).transpose(2, 0, 1))

    s["bq_c"] = col(np.asarray(I["bq"], np.float32) * np.float32(SQ * QSCALE),
                    DC)
    s["bk_c"] = col(I["bk"], DC)
    s["bg_c"] = col(I["bg"], DC)
    s["bo_c"] = col(I["bo"], DC)
    s["bf2_c"] = col(I["bf2"], DC)
    s["bf1_c"] = col(np.asarray(I["bf1"], np.float32) * np.float32(SW1), FC)
    s["bfg_c"] = col(I["bfg"], FC)
    s["ln1_s_c"] = col(I["ln1_s"], DC)
    s["ln1_b_c"] = col(I["ln1_b"], DC)
    s["ln2_s_c"] = col(I["ln2_s"] * sv[:, None], DC)
    s["ln2_b_c"] = col(I["ln2_b"] * sv[:, None], DC)
    s["ln1_s_row"] = _f32(I["ln1_s"][None])
    s["ln2_s_row"] = _f32((I["ln2_s"] * sv[:, None])[None])
    s["bv_row"] = _bf(np.asarray(I["bv"], np.float32) * np.float32(SW))[None]
    s["wd1"] = _bf(np.concatenate(
        [np.asarray(I["Wd1"], np.float32),
         np.zeros((17 * 128 - I["Wd1"].shape[0], 128), np.float32)],
        axis=0).reshape(17, 128, 128).transpose(1, 0, 2))
    s["bd1_c"] = _f32(I["bd1"].reshape(128, 1))
    s["ln3_s_c"] = _f32(I["ln3_s"].reshape(128, 1))
    s["ln3_b_c"] = _f32(I["ln3_b"].reshape(128, 1))
    s["ln3_s_row"] = _f32(I["ln3_s"].reshape(1, 128))
    s["wd2"] = _bf(I["Wd2"])
    s["bd2_c"] = _f32(I["bd2"].reshape(128, 1))
    s["wout"] = _bf(I["Wout"])
    s["bout_t"] = _f32(I["bout"].reshape(1, 1))
    s["onesc"] = np.ones((128, 1), np.float32)
    s["onesr"] = np.ones((1, 128), np.float32)
    return s


def _prep_core(I, shared, c):
    m = dict(shared)
    cgm = np.asarray(I["cgm"], np.float32)
    m["cgmT"] = _bf(cgm[c * BPC:(c + 1) * BPC].reshape(NTOK, DFEAT).T)
    oth = np.asarray(I["other"], np.float32)[c * BPC:(c + 1) * BPC]  # [16,64]
    m["otherT"] = _bf(np.concatenate(
        [oth.T, np.zeros((128 - OTHER, BPC), np.float32)], axis=0))
    # exp of transposed+flipped rel-pos bias table, per-core column window
    rel = np.asarray(I["rel_emb"], np.float32)          # [NL, 255, 128]
    flippedT = rel[:, ::-1, :].transpose(0, 2, 1)       # [NL, 128, 255]
    lo = 112 - 16 * c
    tab = flippedT[:, :, lo:lo + 143]                   # [NL, 128, 143]
    m["exptab"] = _bf(np.exp(tab).transpose(1, 0, 2))   # [128, NL, 143]
    return m


def kernel(**inputs) -> np.ndarray:
    if "nc" not in _cached:
        _cached["nc"] = _build_nc()
    nc = _cached["nc"]
    shared = _prep_shared(inputs)
    in_maps = [_prep_core(inputs, shared, c) for c in range(NCORES)]
    res = run_bass_kernel_spmd(nc, in_maps, core_ids=list(range(NCORES)))
    y = np.concatenate([res.results[c]["y"].reshape(BPC)
                        for c in range(NCORES)])
    return y.reshape(B, 1).astype(np.float32)
